# revision 2
# baseline (speedup 1.0000x reference)
"""BidirectionalAttention TRN2 kernel (optimized).

Data-parallel over batch B=8 across 8 NeuronCores (1 batch element/core).

Per-core algorithm (N=256 tokens, C=768, H=12 heads, D=64):
  - all weights bf16 (host-converted); x bf16 T-layout resident in SBUF
  - qT,kT (T-layout [feat,tok]) bf16; v (N-layout [tok,feat]) bf16; big
    weights (w_qk, w_local, w_global, w_g) preloaded whole with full-row DMAs
  - logitsT[m,n] per head via kT/qT (softmax scale folded into w_q/b_q on host)
  - expT = exp(logitsT) on ACT with row-sum accumulator (rattn normalizer Y)
  - fused attention epilogue matmuls with lhsT=expT: sa_un (rhs=v), Z (rhs=1),
    gs_bias (rhs=giB/Y where giB = gelu-global @ w_bg.T), all bf16
  - wx (per-token generated weights): rhs[(g,c),(k,n)] = gi*li built by one
    DVE bf16 mul per head; the li stream replicates via two [[0,64],...]
    stride-0 DRAM-broadcast DMAs per head (one per partition half), contracted
    by 32 accumulating K=128 matmuls per head against host-permuted w_g;
    wx^T is PE-transposed into the gs PSUM (accumulate) -> isa
  - per-path LayerNorm (bn_stats, bf16) + sigmoid(lam) gating, PE-transpose,
    w_proj
"""
import sys

sys.path.insert(0, "/opt/trn_rl_repo")

import numpy as np
import ml_dtypes
from contextlib import ExitStack

import concourse.bass as bass
import concourse.mybir as mybir
import concourse.tile as tile
from concourse import bacc
from concourse._compat import with_exitstack
from concourse.bass_utils import run_bass_kernel_spmd
from concourse.masks import make_identity

F32 = mybir.dt.float32
BF16 = mybir.dt.bfloat16
AF = mybir.ActivationFunctionType
ALU = mybir.AluOpType

B, N, C, H, D = 8, 256, 768, 12, 64
LN_EPS = 1e-5
NT = N // 128          # token tiles (2)
CC = C // 128          # c-chunks (6)
FT_QK = 2 * C // 128   # q+k feature tiles (12)
NPAIR = D // 2         # 32 (d, d+32) pairs per head
SCALE = D ** -0.5
POOL_ASSIST = ()       # heads whose li A-half replicates via gpsimd instead

_CACHED = {}


def _f32(x):
    return np.ascontiguousarray(np.asarray(x, dtype=np.float32))


def _bf16(x):
    return np.ascontiguousarray(np.asarray(x, dtype=np.float32).astype(ml_dtypes.bfloat16))


@with_exitstack
def _core_kernel(ctx, tc, io, repeat=0):
    nc = tc.nc
    (xT, wqkT, b_qk, wvT, b_v, wglT, b_gl, wloT, b_lo, wg2, wbgT, lam,
     wprT, b_pr, ones_r, li_dram, out) = io

    const = ctx.enter_context(tc.tile_pool(name="const", bufs=1))
    wpool = ctx.enter_context(tc.tile_pool(name="wpool", bufs=3))
    act = ctx.enter_context(tc.tile_pool(name="act", bufs=1))
    work = ctx.enter_context(tc.tile_pool(name="work", bufs=2))
    repool = ctx.enter_context(tc.tile_pool(name="repool", bufs=3))
    opool = ctx.enter_context(tc.tile_pool(name="opool", bufs=1))
    small = ctx.enter_context(tc.tile_pool(name="small", bufs=4))

    # ---------------- constants / resident inputs ----------------
    # critical-path loads first: x, w_local (li gates the broadcast stream)
    xT_b = const.tile([128, CC, N], BF16)           # bf16 x (T-layout)
    nc.sync.dma_start(out=xT_b, in_=xT.rearrange("(cc p) n -> p cc n", p=128))
    wlo_t = const.tile([128, CC, C], BF16)
    nc.sync.dma_start(out=wlo_t, in_=wloT.rearrange("(cc p) f -> p cc f", p=128))
    wgl_t = const.tile([128, CC, C], BF16)
    nc.sync.dma_start(out=wgl_t, in_=wglT.rearrange("(cc p) f -> p cc f", p=128))
    wqk_t = const.tile([128, CC, 2 * C], BF16)
    nc.sync.dma_start(out=wqk_t, in_=wqkT.rearrange("(cc p) f -> p cc f", p=128))
    wg2_t = const.tile([128, NPAIR * D], BF16)
    nc.sync.dma_start(out=wg2_t, in_=wg2)

    # small constants via the Pool/SWDGE queue (keeps HWDGE clear)
    ident_b = const.tile([128, 128], BF16)
    make_identity(nc, ident_b)
    ident_f = const.tile([64, 64], F32)
    make_identity(nc, ident_f)
    ones_b = const.tile([128, 1], BF16)
    nc.vector.memset(ones_b, 1.0)
    eps_t = const.tile([128, 1], F32)
    nc.vector.memset(eps_t, LN_EPS)
    b_lo_t = const.tile([128, CC], F32)
    nc.gpsimd.dma_start(out=b_lo_t, in_=b_lo)
    b_gl_t = const.tile([128, CC], F32)
    nc.gpsimd.dma_start(out=b_gl_t, in_=b_gl)
    b_qk_t = const.tile([128, FT_QK], F32)
    nc.gpsimd.dma_start(out=b_qk_t, in_=b_qk)
    b_v_t = const.tile([128, C], BF16)
    nc.gpsimd.dma_start(out=b_v_t[0:1, :], in_=b_v[None, :])
    b_pr_t = const.tile([128, C], BF16)
    nc.gpsimd.dma_start(out=b_pr_t[0:1, :], in_=b_pr[None, :])
    ones_r_t = const.tile([1, 128], BF16)
    nc.gpsimd.dma_start(out=ones_r_t, in_=ones_r)
    wbg_t = const.tile([128, D], BF16)              # w_bg.T duplicated in halves
    nc.gpsimd.dma_start(out=wbg_t, in_=wbgT)

    lam_t = const.tile([128, 1], F32)
    nc.gpsimd.dma_start(out=lam_t[0:1, :], in_=lam)
    g_row = const.tile([128, 1], F32)
    nc.scalar.activation(out=g_row[0:1, :], in_=lam_t[0:1, :], func=AF.Sigmoid)
    g_t = const.tile([128, 1], F32)
    nc.gpsimd.partition_broadcast(out_ap=g_t, in_ap=g_row[0:1, :])
    gm1_t = const.tile([128, 1], F32)
    nc.scalar.activation(out=gm1_t, in_=g_t, func=AF.Identity, bias=1.0, scale=-1.0)

    # ---------------- PSUM pools (whole-kernel, 8 banks) ----------------
    psA = ctx.enter_context(tc.tile_pool(name="psA", bufs=2, space="PSUM"))
    psB = ctx.enter_context(tc.tile_pool(name="psB", bufs=1, space="PSUM"))
    psC = ctx.enter_context(tc.tile_pool(name="psC", bufs=1, space="PSUM"))

    def pst(tag, shape):
        if tag == "tp":
            t = psA.tile([128, 256], F32, tag="mm256")
            return t.bitcast(BF16)[:, 0:shape[1]]
        if tag in ("qk", "gl", "lg"):
            t = psA.tile([128, 256], F32, tag="mm256")
        elif tag in ("v0", "pr0"):
            t = psB.tile([128, 512], F32, tag="mm512a")
        elif tag in ("v1", "pr1"):
            t = psB.tile([128, 512], F32, tag="mm512b")
        else:
            t = psC.tile([128, {"wx": 256, "sa": D, "z": 1, "gs": D}[tag]], F32, tag=tag)
        if list(t.shape) == list(shape):
            return t
        return t[:, 0:shape[1]]

    def body():
        # ---------------- phase 3a: liT (gelu, bf16) -> DRAM stream ----------------
        gi_t = act.tile([128, CC, N], BF16)
        li_t = act.tile([128, CC, N], BF16)
        for ft in range(CC):
            mm = pst("gl", [128, N])
            for cc in range(CC):
                nc.tensor.matmul(mm, wlo_t[:, cc, ft * 128:(ft + 1) * 128],
                                 xT_b[:, cc, :],
                                 start=(cc == 0), stop=(cc == CC - 1))
            nc.scalar.activation(out=li_t[:, ft, :], in_=mm, func=AF.Gelu,
                                 bias=b_lo_t[:, ft:ft + 1])
            # li streams to DRAM: li_dram[h, d*N+n]
            nc.sync.dma_start(
                out=li_dram[2 * ft:2 * ft + 2, :].rearrange("h (d n) -> (h d) n", n=N),
                in_=li_t[:, ft, :])

        # ---------------- phase 3b: giT ----------------
        gi_hh = act.tile([128, H, N], BF16)
        for ft in range(CC):
            mm = pst("gl", [128, N])
            for cc in range(CC):
                nc.tensor.matmul(mm, wgl_t[:, cc, ft * 128:(ft + 1) * 128],
                                 xT_b[:, cc, :],
                                 start=(cc == 0), stop=(cc == CC - 1))
            nc.scalar.activation(out=gi_t[:, ft, :], in_=mm, func=AF.Gelu,
                                 bias=b_gl_t[:, ft:ft + 1])
            # gi head-stacked: gi_hh[:, h, :] = [gi_h(c); gi_h(c)]
            for half in range(2):
                h = 2 * ft + half
                nc.sync.dma_start(out=gi_hh[0:64, h, :],
                                  in_=gi_t[half * 64:half * 64 + 64, ft, :])
                nc.sync.dma_start(out=gi_hh[64:128, h, :],
                                  in_=gi_t[half * 64:half * 64 + 64, ft, :])

        # ---------------- phase 1: qT / kT (bf16) ----------------
        qk_t = act.tile([128, FT_QK, N], BF16)
        for ft in range(FT_QK):
            mm = pst("qk", [128, N])
            for cc in range(CC):
                nc.tensor.matmul(mm, wqk_t[:, cc, ft * 128:(ft + 1) * 128],
                                 xT_b[:, cc, :],
                                 start=(cc == 0), stop=(cc == CC - 1))
            nc.scalar.activation(out=qk_t[:, ft, :], in_=mm, func=AF.Identity,
                                 bias=b_qk_t[:, ft:ft + 1])

        # ---------------- phase 2: v (N-layout, bf16) ----------------
        v_t = act.tile([128, NT, C], BF16)
        vps = [pst("v0", [128, 512]), pst("v1", [128, 512])]
        for lo, hi in ((0, 512), (512, 768)):
            gw = hi - lo
            for cc in range(CC):
                w_t = wpool.tile([128, 512], BF16, tag="wv")
                nc.sync.dma_start(
                    out=w_t[:, 0:gw],
                    in_=wvT.rearrange("(cc p) f -> p cc f", p=128)[:, cc, lo:hi])
                for nt in range(NT):
                    nc.tensor.matmul(vps[nt][:, 0:gw],
                                     xT_b[:, cc, nt * 128:(nt + 1) * 128],
                                     w_t[:, 0:gw], start=(cc == 0), stop=False)
            for nt in range(NT):
                nc.tensor.matmul(vps[nt][:, 0:gw], ones_r_t,
                                 b_v_t[0:1, lo:hi], start=False, stop=True)
                nc.scalar.copy(out=v_t[:, nt, lo:hi], in_=vps[nt][:, 0:gw])

        # ---------------- phase 4: attention + generated weights ----------------
        sa_sb = act.tile([128, NT, C], BF16)
        isa_sb = act.tile([128, NT, C], BF16)

        for h in range(H):
            ft, half = h // 2, h % 2
            base = half * 64
            # logitsT + exp + Y
            exp_h = work.tile([128, 2, N], BF16, tag="exp")
            recipY = small.tile([128, 2], F32, tag="recipY")
            for mt in range(2):
                lg = pst("lg", [128, N])
                nc.tensor.matmul(
                    lg,
                    qk_t[base:base + 64, FT_QK // 2 + ft, mt * 128:(mt + 1) * 128],
                    qk_t[base:base + 64, ft, :],
                    start=True, stop=True)
                ysum = small.tile([128, 1], F32, tag="ysum")
                nc.scalar.activation(out=exp_h[:, mt, :], in_=lg, func=AF.Exp,
                                     accum_out=ysum)
                nc.vector.reciprocal(out=recipY[:, mt:mt + 1], in_=ysum)

            # giB = giT_h^T @ w_bg.T, scaled by 1/Y
            giBY = work.tile([128, 2, D], BF16, tag="giBY")
            for mt in range(2):
                gb = pst("lg", [128, D])
                nc.tensor.matmul(gb, gi_t[base:base + 64, ft, mt * 128:(mt + 1) * 128],
                                 wbg_t[base:base + 64, :], start=True, stop=True)
                nc.scalar.activation(out=giBY[:, mt, :], in_=gb, func=AF.Copy,
                                     scale=recipY[:, mt:mt + 1])

            # li replication into li_rep [128, 8192]: two stride-0
            # DRAM-broadcast DMAs (one per partition half); POOL_ASSIST
            # heads replicate the A-half via gpsimd partition_broadcast
            li_rep = repool.tile([128, NPAIR * N], BF16, tag="lirep")
            row_step = li_dram.ap[0][0]
            if h in POOL_ASSIST:
                nc.sync.dma_start(out=li_rep[0:1, :],
                                  in_=li_dram[h:h + 1, 0:NPAIR * N])
                nc.gpsimd.partition_broadcast(out_ap=li_rep[0:64, :],
                                              in_ap=li_rep[0:1, :], channels=64)
                gs_ = (1,)
            else:
                gs_ = (0, 1)
            for g in gs_:
                srcg = bass.AP(tensor=li_dram.tensor,
                               offset=(li_dram.offset + h * row_step
                                       + g * NPAIR * N),
                               ap=[[0, 64], [1, NPAIR * N]])
                nc.sync.dma_start(out=li_rep[g * 64:(g + 1) * 64, :], in_=srcg)
            gi_rep = bass.AP(tensor=gi_hh.tensor,
                             offset=gi_hh.offset + gi_hh.ap[1][0] * h,
                             ap=[gi_hh.ap[0], [0, NPAIR], [1, N]])
            rhs = work.tile([128, NPAIR * N], BF16, tag="rhs")
            nc.vector.tensor_tensor(out=rhs, in0=gi_rep, in1=li_rep, op=ALU.mult)

            # wx accumulation -> wxT [e, n]
            wx_full = pst("wx", [128, N])
            wx_ps = wx_full[0:64, :]
            for k in range(NPAIR):
                nc.tensor.matmul(wx_ps, wg2_t[:, k * D:(k + 1) * D],
                                 rhs[:, k * N:(k + 1) * N],
                                 start=(k == 0), stop=(k == NPAIR - 1))
            wx_sbf = work.tile([128, N], F32, tag="wxsb")
            wx_sb = wx_sbf[0:64, :]
            nc.scalar.copy(out=wx_sb, in_=wx_ps)

            # fused epilogue per n-tile
            for nt in range(NT):
                sa_ps = pst("sa", [128, D])
                z_ps = pst("z", [128, 1])
                gs_ps = pst("gs", [128, D])
                for mt in range(2):
                    lhs = exp_h[:, mt, nt * 128:(nt + 1) * 128]
                    nc.tensor.matmul(sa_ps, lhs, v_t[:, mt, h * D:(h + 1) * D],
                                     start=(mt == 0), stop=(mt == 1))
                    nc.tensor.matmul(z_ps, lhs, ones_b,
                                     start=(mt == 0), stop=(mt == 1))
                    nc.tensor.matmul(gs_ps, lhs, giBY[:, mt, :],
                                     start=(mt == 0), stop=False)
                nc.tensor.matmul(gs_ps, wx_sb[:, nt * 128:(nt + 1) * 128],
                                 ident_f,
                                 is_transpose=True, start=False, stop=True)
                recipZ = small.tile([128, 1], F32, tag="recipZ")
                nc.vector.reciprocal(out=recipZ, in_=z_ps)
                nc.vector.tensor_scalar_mul(out=sa_sb[:, nt, h * D:(h + 1) * D],
                                            in0=sa_ps, scalar1=recipZ)
                nc.scalar.copy(out=isa_sb[:, nt, h * D:(h + 1) * D], in_=gs_ps)

        # ---------------- phase 5: LayerNorm + mix + proj ----------------
        out_nl = out.rearrange("(nt p) c -> p nt c", p=128)
        mixT = work.tile([128, NT, CC, 128], BF16, tag="mixT")
        for nt in range(NT):
            mix = work.tile([128, C], BF16, tag="mix")
            scratch = work.tile([128, C], BF16, tag="scratch")
            for src_t, gate, accum in ((sa_sb, g_t, False), (isa_sb, gm1_t, True)):
                stats = small.tile([128, 3, nc.vector.BN_STATS_DIM], F32, tag="st")
                for s in range(3):
                    nc.vector.bn_stats(out=stats[:, s, :],
                                       in_=src_t[:, nt, s * 256:(s + 1) * 256])
                mv = small.tile([128, nc.vector.BN_AGGR_DIM], F32, tag="mv")
                nc.vector.bn_aggr(out=mv, in_=stats)
                rstd = small.tile([128, 1], F32, tag="rstd")
                nc.scalar.activation(out=rstd, in_=mv[:, 1:2], func=AF.Sqrt, bias=eps_t)
                nc.vector.reciprocal(out=rstd, in_=rstd)
                nc.vector.tensor_tensor(out=rstd, in0=rstd, in1=gate, op=ALU.mult)
                dst = scratch if accum else mix
                nc.vector.tensor_scalar(out=dst, in0=src_t[:, nt, :],
                                        scalar1=mv[:, 0:1], scalar2=rstd,
                                        op0=ALU.subtract, op1=ALU.mult)
                if accum:
                    nc.vector.tensor_tensor(out=mix, in0=mix, in1=scratch, op=ALU.add)
            for cc in range(CC):
                tp = pst("tp", [128, 128])
                nc.tensor.matmul(tp, mix[:, cc * 128:(cc + 1) * 128], ident_b,
                                 is_transpose=True, start=True, stop=True)
                nc.scalar.copy(out=mixT[:, nt, cc, :], in_=tp)
        prps = [pst("pr0", [128, 512]), pst("pr1", [128, 512])]
        out_sb = opool.tile([128, NT, C], F32, tag="outsb")
        for lo, hi in ((0, 512), (512, 768)):
            gw = hi - lo
            for cc in range(CC):
                w_t = wpool.tile([128, 512], BF16, tag="wpr")
                nc.sync.dma_start(
                    out=w_t[:, 0:gw],
                    in_=wprT.rearrange("(cc p) f -> p cc f", p=128)[:, cc, lo:hi])
                for nt in range(NT):
                    nc.tensor.matmul(prps[nt][:, 0:gw], mixT[:, nt, cc, :],
                                     w_t[:, 0:gw], start=(cc == 0), stop=False)
            for nt in range(NT):
                nc.tensor.matmul(prps[nt][:, 0:gw], ones_r_t,
                                 b_pr_t[0:1, lo:hi], start=False, stop=True)
                nc.scalar.copy(out=out_sb[:, nt, lo:hi], in_=prps[nt][:, 0:gw])
        for nt in range(NT):
            nc.sync.dma_start(out=out_nl[:, nt, :], in_=out_sb[:, nt, :])

    if repeat:
        with tc.For_i(0, repeat, 1):
            body()
    else:
        body()


def _build(repeat=0):
    nc = bacc.Bacc("TRN2", target_bir_lowering=False, debug=False, num_devices=8)

    def inp(name, shape, dtype=F32):
        return nc.dram_tensor(name, list(shape), dtype, kind="ExternalInput").ap()

    io = [
        inp("xT", (C, N), BF16),
        inp("wqkT", (C, 2 * C), BF16),
        inp("b_qk", (128, FT_QK)),
        inp("wvT", (C, C), BF16),
        inp("b_v", (C,), BF16),
        inp("wglT", (C, C), BF16),
        inp("b_gl", (128, CC)),
        inp("wloT", (C, C), BF16),
        inp("b_lo", (128, CC)),
        inp("wg2", (128, NPAIR * D), BF16),
        inp("wbgT", (128, D), BF16),
        inp("lam", (1, 1)),
        inp("wprT", (C, C), BF16),
        inp("b_pr", (C,), BF16),
        inp("ones_r", (1, 128), BF16),
        nc.dram_tensor("li_dram", [H, D * N], BF16).ap(),   # internal scratch
        nc.dram_tensor("out", [N, C], F32, kind="ExternalOutput").ap(),
    ]
    with tile.TileContext(nc) as tc:
        _core_kernel(tc, io, repeat=repeat)
    nc.compile()
    return nc


def kernel(**inputs):
    x = _f32(inputs["x"])
    w_qkv = _f32(inputs["w_qkv"]); b_qkv = _f32(inputs["b_qkv"])
    w_g = _f32(inputs["w_g"]); w_bg = _f32(inputs["w_bg"])
    w_local = _f32(inputs["w_local"]); b_local = _f32(inputs["b_local"])
    w_global = _f32(inputs["w_global"]); b_global = _f32(inputs["b_global"])
    lam = _f32(inputs["lam"])
    w_proj = _f32(inputs["w_proj"]); b_proj = _f32(inputs["b_proj"])

    wq = w_qkv[0:C] * SCALE
    wk = w_qkv[C:2 * C]
    wv = w_qkv[2 * C:3 * C]
    bq = b_qkv[0:C] * SCALE
    bk = b_qkv[C:2 * C]
    bv = b_qkv[2 * C:3 * C]
    wqkT = _bf16(np.concatenate([wq, wk], 0).T)
    b_qk = _f32(np.concatenate([bq, bk]).reshape(FT_QK, 128).T)
    wvT = _bf16(wv.T)
    wglT = _bf16(w_global.T)
    b_gl = _f32(b_global.reshape(CC, 128).T)
    wloT = _bf16(w_local.T)
    b_lo = _f32(b_local.reshape(CC, 128).T)
    wprT = _bf16(w_proj.T)
    wg3 = w_g.reshape(D, D, D)                # [d, e, c]
    wg2 = np.zeros((128, NPAIR * D), np.float32)
    for k in range(NPAIR):
        wg2[0:64, k * D:(k + 1) * D] = wg3[k].T
        wg2[64:128, k * D:(k + 1) * D] = wg3[k + NPAIR].T
    wg2 = _bf16(wg2)
    wbgT = _bf16(np.concatenate([w_bg.T, w_bg.T], 0))   # duplicated halves

    if "nc" not in _CACHED:
        _CACHED["nc"] = _build()
    nc = _CACHED["nc"]

    shared = dict(wqkT=wqkT, b_qk=b_qk, wvT=wvT, b_v=_bf16(bv), wglT=wglT, b_gl=b_gl,
                  wloT=wloT, b_lo=b_lo, wg2=wg2, wbgT=wbgT,
                  lam=lam.reshape(1, 1), wprT=wprT, b_pr=_bf16(b_proj),
                  ones_r=_bf16(np.ones((1, 128), np.float32)))
    in_maps = [dict(shared, xT=_bf16(x[b].T)) for b in range(B)]
    _CACHED["in_maps"] = in_maps
    res = run_bass_kernel_spmd(nc, in_maps, core_ids=list(range(B)))
    out = np.stack([res.results[b]["out"] for b in range(B)], 0)
    return out.astype(np.float32)


def _device_runner(nc, in_maps):
    """Single-bind sharded jitted fn with device-resident inputs."""
    import jax
    from jax.sharding import Mesh, PartitionSpec
    from jax.experimental.shard_map import shard_map
    import concourse.mybir as _mb
    from concourse import bass2jax as B2J

    B2J.install_neuronx_cc_hook()
    partition_name = nc.partition_id_tensor.name if nc.partition_id_tensor else None
    in_names, out_names, out_avals, zero_outs = [], [], [], []
    for alloc in nc.m.functions[0].allocations:
        if not isinstance(alloc, _mb.MemoryLocationSet):
            continue
        name = alloc.memorylocations[0].name
        if alloc.kind == "ExternalInput":
            if name != partition_name:
                in_names.append(name)
        elif alloc.kind == "ExternalOutput":
            shape = tuple(alloc.tensor_shape)
            dtype = _mb.dt.np(alloc.dtype)
            out_names.append(name)
            out_avals.append(jax.core.ShapedArray(shape, dtype))
            zero_outs.append(np.zeros(shape, dtype))
    n_params = len(in_names)
    all_in_names = list(in_names) + list(out_names)
    if partition_name is not None:
        all_in_names.append(partition_name)

    def _body(*args):
        operands = list(args)
        if partition_name is not None:
            operands.append(B2J.partition_id_tensor())
        return tuple(B2J._bass_exec_p.bind(
            *operands,
            out_avals=tuple(out_avals),
            in_names=tuple(all_in_names),
            out_names=tuple(out_names),
            lowering_input_output_aliases=(),
            sim_require_finite=True,
            sim_require_nnan=True,
            nc=nc,
        ))

    n_cores = len(in_maps)
    devices = jax.devices()[:n_cores]
    mesh = Mesh(np.asarray(devices), ("core",))
    n_outs = len(out_avals)
    sharded = jax.jit(shard_map(
        _body, mesh=mesh,
        in_specs=(PartitionSpec("core"),) * (n_params + n_outs),
        out_specs=(PartitionSpec("core"),) * n_outs, check_rep=False))
    per_core = [[np.asarray(m[nm]) for nm in in_names] for m in in_maps]
    concat_in = [np.concatenate([per_core[c][i] for c in range(n_cores)], 0)
                 for i in range(n_params)]
    concat_zeros = [np.zeros((n_cores * z.shape[0], *z.shape[1:]), z.dtype)
                    for z in zero_outs]
    dev_in = [jax.device_put(a) for a in concat_in]
    dev_zero = [jax.device_put(a) for a in concat_zeros]

    def run():
        return sharded(*dev_in, *dev_zero)

    return run


def run_timed(n_iters=1024, n_calls=12, **inputs):
    """Build repeat=1 and repeat=n_iters variants; time both with
    device-resident inputs; return estimated per-iteration ns
    (median-based; min-based printed for reference)."""
    import time
    import statistics
    import jax
    kernel(**inputs)
    in_maps = _CACHED["in_maps"]
    wmed, wmin = {}, {}
    for R in (1, n_iters):
        key = f"nc_rep{R}"
        if key not in _CACHED:
            _CACHED[key] = _build(repeat=R)
        run = _device_runner(_CACHED[key], in_maps)
        jax.block_until_ready(run())  # compile+warm
        ts = []
        for _ in range(n_calls):
            t0 = time.time()
            jax.block_until_ready(run())
            ts.append(time.time() - t0)
        wmed[R], wmin[R] = statistics.median(ts), min(ts)
        print(f"repeat={R}: med {wmed[R]*1e3:.2f} min {wmin[R]*1e3:.2f} ms "
              f"(all {[f'{t*1e3:.1f}' for t in ts]})")
    per_med = (wmed[n_iters] - wmed[1]) / (n_iters - 1) * 1e9
    per_min = (wmin[n_iters] - wmin[1]) / (n_iters - 1) * 1e9
    print(f"per-iter median-est {per_med:.0f} ns / min-est {per_min:.0f} ns")
    return per_med


# revision 3
# speedup vs baseline: 1.0923x; 1.0923x over previous
"""BidirectionalAttention TRN2 kernel (optimized).

Data-parallel over batch B=8 across 8 NeuronCores (1 batch element/core).

Per-core algorithm (N=256 tokens, C=768, H=12 heads, D=64):
  - all weights bf16 (host-converted); x bf16 T-layout resident in SBUF
  - qT,kT (T-layout [feat,tok]) bf16; v (N-layout [tok,feat]) bf16; big
    weights (w_qk, w_local, w_global, w_g) preloaded whole with full-row DMAs
  - logitsT[m,n] per head via kT/qT (softmax scale folded into w_q/b_q on host)
  - expT = exp(logitsT) on ACT with row-sum accumulator (rattn normalizer Y)
  - fused attention epilogue matmuls with lhsT=expT: sa_un (rhs=v), Z (rhs=1),
    gs_bias (rhs=giB/Y where giB = gelu-global @ w_bg.T), all bf16
  - wx (per-token generated weights): rhs[(g,c),(k,n)] = gi*li built by one
    DVE bf16 mul per head; the li stream replicates via two [[0,64],...]
    stride-0 DRAM-broadcast DMAs per head (one per partition half), contracted
    by 32 accumulating K=128 matmuls per head against host-permuted w_g;
    wx^T is PE-transposed into the gs PSUM (accumulate) -> isa
  - per-path LayerNorm (bn_stats, bf16) + sigmoid(lam) gating, PE-transpose,
    w_proj
"""
import sys

sys.path.insert(0, "/opt/trn_rl_repo")

import numpy as np
import ml_dtypes
from contextlib import ExitStack

import concourse.bass as bass
import concourse.mybir as mybir
import concourse.tile as tile
from concourse import bacc
from concourse._compat import with_exitstack
from concourse.bass_utils import run_bass_kernel_spmd
from concourse.masks import make_identity

F32 = mybir.dt.float32
BF16 = mybir.dt.bfloat16
AF = mybir.ActivationFunctionType
ALU = mybir.AluOpType

B, N, C, H, D = 8, 256, 768, 12, 64
LN_EPS = 1e-5
NT = N // 128          # token tiles (2)
CC = C // 128          # c-chunks (6)
FT_QK = 2 * C // 128   # q+k feature tiles (12)
NPAIR = D // 2         # 32 (d, d+32) pairs per head
SCALE = D ** -0.5
POOL_ASSIST = ()       # heads whose li A-half replicates via gpsimd instead

_CACHED = {}


def _f32(x):
    return np.ascontiguousarray(np.asarray(x, dtype=np.float32))


def _bf16(x):
    return np.ascontiguousarray(np.asarray(x, dtype=np.float32).astype(ml_dtypes.bfloat16))


@with_exitstack
def _core_kernel(ctx, tc, io, repeat=0):
    nc = tc.nc
    (xT, wqkT, b_qk, wvT, b_v, wglT, b_gl, wloT, b_lo, wg2, wbgT, lam,
     wprT, b_pr, ones_r, li_dram, out) = io

    const = ctx.enter_context(tc.tile_pool(name="const", bufs=1))
    wpool = ctx.enter_context(tc.tile_pool(name="wpool", bufs=3))
    act = ctx.enter_context(tc.tile_pool(name="act", bufs=1))
    work = ctx.enter_context(tc.tile_pool(name="work", bufs=2))
    repool = ctx.enter_context(tc.tile_pool(name="repool", bufs=3))
    opool = ctx.enter_context(tc.tile_pool(name="opool", bufs=1))
    small = ctx.enter_context(tc.tile_pool(name="small", bufs=4))

    # ---------------- constants / resident inputs ----------------
    # critical-path loads first: x, w_local (li gates the broadcast stream)
    xT_b = const.tile([128, CC, N], BF16)           # bf16 x (T-layout)
    nc.sync.dma_start(out=xT_b, in_=xT.rearrange("(cc p) n -> p cc n", p=128))
    wlo_t = const.tile([128, CC, C], BF16)
    nc.sync.dma_start(out=wlo_t, in_=wloT.rearrange("(cc p) f -> p cc f", p=128))
    wgl_t = const.tile([128, CC, C], BF16)
    nc.sync.dma_start(out=wgl_t, in_=wglT.rearrange("(cc p) f -> p cc f", p=128))
    wqk_t = const.tile([128, CC, 2 * C], BF16)
    nc.sync.dma_start(out=wqk_t, in_=wqkT.rearrange("(cc p) f -> p cc f", p=128))
    wg2_t = const.tile([128, NPAIR * D], BF16)
    nc.sync.dma_start(out=wg2_t, in_=wg2)

    # small constants via the Pool/SWDGE queue (keeps HWDGE clear)
    ident_b = const.tile([128, 128], BF16)
    make_identity(nc, ident_b)
    ident_f = const.tile([64, 64], F32)
    make_identity(nc, ident_f)
    ones_b = const.tile([128, 1], BF16)
    nc.vector.memset(ones_b, 1.0)
    eps_t = const.tile([128, 1], F32)
    nc.vector.memset(eps_t, LN_EPS)
    b_lo_t = const.tile([128, CC], F32)
    nc.gpsimd.dma_start(out=b_lo_t, in_=b_lo)
    b_gl_t = const.tile([128, CC], F32)
    nc.gpsimd.dma_start(out=b_gl_t, in_=b_gl)
    b_qk_t = const.tile([128, FT_QK], F32)
    nc.gpsimd.dma_start(out=b_qk_t, in_=b_qk)
    b_v_t = const.tile([128, C], BF16)
    nc.gpsimd.dma_start(out=b_v_t[0:1, :], in_=b_v[None, :])
    b_pr_t = const.tile([128, C], BF16)
    nc.gpsimd.dma_start(out=b_pr_t[0:1, :], in_=b_pr[None, :])
    ones_r_t = const.tile([1, 128], BF16)
    nc.gpsimd.dma_start(out=ones_r_t, in_=ones_r)
    wbg_t = const.tile([128, D], BF16)              # w_bg.T duplicated in halves
    nc.gpsimd.dma_start(out=wbg_t, in_=wbgT)

    lam_t = const.tile([128, 1], F32)
    nc.gpsimd.dma_start(out=lam_t[0:1, :], in_=lam)
    g_row = const.tile([128, 1], F32)
    nc.scalar.activation(out=g_row[0:1, :], in_=lam_t[0:1, :], func=AF.Sigmoid)
    g_t = const.tile([128, 1], F32)
    nc.gpsimd.partition_broadcast(out_ap=g_t, in_ap=g_row[0:1, :])
    gm1_t = const.tile([128, 1], F32)
    nc.scalar.activation(out=gm1_t, in_=g_t, func=AF.Identity, bias=1.0, scale=-1.0)

    # ---------------- PSUM pools (whole-kernel, 8 banks) ----------------
    psA = ctx.enter_context(tc.tile_pool(name="psA", bufs=2, space="PSUM"))
    psB = ctx.enter_context(tc.tile_pool(name="psB", bufs=1, space="PSUM"))
    psC = ctx.enter_context(tc.tile_pool(name="psC", bufs=1, space="PSUM"))

    def pst(tag, shape):
        if tag == "tp":
            t = psA.tile([128, 256], F32, tag="mm256")
            return t.bitcast(BF16)[:, 0:shape[1]]
        if tag in ("qk", "gl", "lg"):
            t = psA.tile([128, 256], F32, tag="mm256")
        elif tag in ("v0", "pr0"):
            t = psB.tile([128, 512], F32, tag="mm512a")
        elif tag in ("v1", "pr1"):
            t = psB.tile([128, 512], F32, tag="mm512b")
        else:
            t = psC.tile([128, {"wx": 256, "sa": D, "z": 1, "gs": D}[tag]], F32, tag=tag)
        if list(t.shape) == list(shape):
            return t
        return t[:, 0:shape[1]]

    def body():
        # ---------------- phase 3a: liT (gelu, bf16) -> DRAM stream ----------------
        gi_t = act.tile([128, CC, N], BF16)
        li_t = act.tile([128, CC, N], BF16)
        for ft in range(CC):
            mm = pst("gl", [128, N])
            for cc in range(CC):
                nc.tensor.matmul(mm, wlo_t[:, cc, ft * 128:(ft + 1) * 128],
                                 xT_b[:, cc, :],
                                 start=(cc == 0), stop=(cc == CC - 1))
            nc.scalar.activation(out=li_t[:, ft, :], in_=mm, func=AF.Gelu,
                                 bias=b_lo_t[:, ft:ft + 1])
            # li streams to DRAM: li_dram[h, d*N+n]
            nc.sync.dma_start(
                out=li_dram[2 * ft:2 * ft + 2, :].rearrange("h (d n) -> (h d) n", n=N),
                in_=li_t[:, ft, :])

        # ---------------- phase 3b: giT ----------------
        gi_hh = act.tile([128, H, N], BF16)
        for ft in range(CC):
            mm = pst("gl", [128, N])
            for cc in range(CC):
                nc.tensor.matmul(mm, wgl_t[:, cc, ft * 128:(ft + 1) * 128],
                                 xT_b[:, cc, :],
                                 start=(cc == 0), stop=(cc == CC - 1))
            nc.scalar.activation(out=gi_t[:, ft, :], in_=mm, func=AF.Gelu,
                                 bias=b_gl_t[:, ft:ft + 1])
            # gi head-stacked: gi_hh[:, h, :] = [gi_h(c); gi_h(c)]
            for half in range(2):
                h = 2 * ft + half
                nc.sync.dma_start(out=gi_hh[0:64, h, :],
                                  in_=gi_t[half * 64:half * 64 + 64, ft, :])
                nc.sync.dma_start(out=gi_hh[64:128, h, :],
                                  in_=gi_t[half * 64:half * 64 + 64, ft, :])

        # ---------------- phase 1: qT / kT (bf16) ----------------
        qk_t = act.tile([128, FT_QK, N], BF16)
        for ft in range(FT_QK):
            mm = pst("qk", [128, N])
            for cc in range(CC):
                nc.tensor.matmul(mm, wqk_t[:, cc, ft * 128:(ft + 1) * 128],
                                 xT_b[:, cc, :],
                                 start=(cc == 0), stop=(cc == CC - 1))
            nc.scalar.activation(out=qk_t[:, ft, :], in_=mm, func=AF.Identity,
                                 bias=b_qk_t[:, ft:ft + 1])

        # ---------------- phase 2: v (N-layout, bf16) ----------------
        v_t = act.tile([128, NT, C], BF16)
        vps = [pst("v0", [128, 512]), pst("v1", [128, 512])]
        for lo, hi in ((0, 512), (512, 768)):
            gw = hi - lo
            for cc in range(CC):
                w_t = wpool.tile([128, 512], BF16, tag="wv")
                nc.sync.dma_start(
                    out=w_t[:, 0:gw],
                    in_=wvT.rearrange("(cc p) f -> p cc f", p=128)[:, cc, lo:hi])
                for nt in range(NT):
                    nc.tensor.matmul(vps[nt][:, 0:gw],
                                     xT_b[:, cc, nt * 128:(nt + 1) * 128],
                                     w_t[:, 0:gw], start=(cc == 0), stop=False)
            for nt in range(NT):
                nc.tensor.matmul(vps[nt][:, 0:gw], ones_r_t,
                                 b_v_t[0:1, lo:hi], start=False, stop=True)
                nc.scalar.copy(out=v_t[:, nt, lo:hi], in_=vps[nt][:, 0:gw])

        # ---------------- phase 4: attention + generated weights ----------------
        sa_sb = act.tile([128, NT, C], BF16)
        isa_sb = act.tile([128, NT, C], BF16)

        for h in range(H):
            ft, half = h // 2, h % 2
            base = half * 64
            # logitsT + exp + Y
            exp_h = work.tile([128, 2, N], BF16, tag="exp")
            recipY = small.tile([128, 2], F32, tag="recipY")
            for mt in range(2):
                lg = pst("lg", [128, N])
                nc.tensor.matmul(
                    lg,
                    qk_t[base:base + 64, FT_QK // 2 + ft, mt * 128:(mt + 1) * 128],
                    qk_t[base:base + 64, ft, :],
                    start=True, stop=True)
                ysum = small.tile([128, 1], F32, tag="ysum")
                nc.scalar.activation(out=exp_h[:, mt, :], in_=lg, func=AF.Exp,
                                     accum_out=ysum)
                nc.vector.reciprocal(out=recipY[:, mt:mt + 1], in_=ysum)

            # giB = giT_h^T @ w_bg.T, scaled by 1/Y
            giBY = work.tile([128, 2, D], BF16, tag="giBY")
            for mt in range(2):
                gb = pst("lg", [128, D])
                nc.tensor.matmul(gb, gi_t[base:base + 64, ft, mt * 128:(mt + 1) * 128],
                                 wbg_t[base:base + 64, :], start=True, stop=True)
                nc.scalar.activation(out=giBY[:, mt, :], in_=gb, func=AF.Copy,
                                     scale=recipY[:, mt:mt + 1])

            # li replication into li_rep [128, 8192]: two stride-0
            # DRAM-broadcast DMAs (one per partition half); POOL_ASSIST
            # heads replicate the A-half via gpsimd partition_broadcast
            li_rep = repool.tile([128, NPAIR * N], BF16, tag="lirep")
            row_step = li_dram.ap[0][0]
            if h in POOL_ASSIST:
                nc.sync.dma_start(out=li_rep[0:1, :],
                                  in_=li_dram[h:h + 1, 0:NPAIR * N])
                nc.gpsimd.partition_broadcast(out_ap=li_rep[0:64, :],
                                              in_ap=li_rep[0:1, :], channels=64)
                gs_ = (1,)
            else:
                gs_ = (0, 1)
            for g in gs_:
                srcg = bass.AP(tensor=li_dram.tensor,
                               offset=(li_dram.offset + h * row_step
                                       + g * NPAIR * N),
                               ap=[[0, 64], [1, NPAIR * N]])
                # split the two halves across the SP and ACT hardware DGE
                # queues so they transfer concurrently
                eng = nc.scalar if g == 1 else nc.sync
                eng.dma_start(out=li_rep[g * 64:(g + 1) * 64, :], in_=srcg)
            gi_rep = bass.AP(tensor=gi_hh.tensor,
                             offset=gi_hh.offset + gi_hh.ap[1][0] * h,
                             ap=[gi_hh.ap[0], [0, NPAIR], [1, N]])
            rhs = work.tile([128, NPAIR * N], BF16, tag="rhs")
            nc.vector.tensor_tensor(out=rhs, in0=gi_rep, in1=li_rep, op=ALU.mult)

            # wx accumulation -> wxT [e, n]
            wx_full = pst("wx", [128, N])
            wx_ps = wx_full[0:64, :]
            for k in range(NPAIR):
                nc.tensor.matmul(wx_ps, wg2_t[:, k * D:(k + 1) * D],
                                 rhs[:, k * N:(k + 1) * N],
                                 start=(k == 0), stop=(k == NPAIR - 1))
            wx_sbf = work.tile([128, N], F32, tag="wxsb")
            wx_sb = wx_sbf[0:64, :]
            nc.scalar.copy(out=wx_sb, in_=wx_ps)

            # fused epilogue per n-tile
            for nt in range(NT):
                sa_ps = pst("sa", [128, D])
                z_ps = pst("z", [128, 1])
                gs_ps = pst("gs", [128, D])
                for mt in range(2):
                    lhs = exp_h[:, mt, nt * 128:(nt + 1) * 128]
                    nc.tensor.matmul(sa_ps, lhs, v_t[:, mt, h * D:(h + 1) * D],
                                     start=(mt == 0), stop=(mt == 1))
                    nc.tensor.matmul(z_ps, lhs, ones_b,
                                     start=(mt == 0), stop=(mt == 1))
                    nc.tensor.matmul(gs_ps, lhs, giBY[:, mt, :],
                                     start=(mt == 0), stop=False)
                nc.tensor.matmul(gs_ps, wx_sb[:, nt * 128:(nt + 1) * 128],
                                 ident_f,
                                 is_transpose=True, start=False, stop=True)
                recipZ = small.tile([128, 1], F32, tag="recipZ")
                nc.vector.reciprocal(out=recipZ, in_=z_ps)
                nc.vector.tensor_scalar_mul(out=sa_sb[:, nt, h * D:(h + 1) * D],
                                            in0=sa_ps, scalar1=recipZ)
                nc.scalar.copy(out=isa_sb[:, nt, h * D:(h + 1) * D], in_=gs_ps)

        # ---------------- phase 5: LayerNorm + mix + proj ----------------
        out_nl = out.rearrange("(nt p) c -> p nt c", p=128)
        mixT = work.tile([128, NT, CC, 128], BF16, tag="mixT")
        for nt in range(NT):
            mix = work.tile([128, C], BF16, tag="mix")
            scratch = work.tile([128, C], BF16, tag="scratch")
            for src_t, gate, accum in ((sa_sb, g_t, False), (isa_sb, gm1_t, True)):
                stats = small.tile([128, 3, nc.vector.BN_STATS_DIM], F32, tag="st")
                for s in range(3):
                    nc.vector.bn_stats(out=stats[:, s, :],
                                       in_=src_t[:, nt, s * 256:(s + 1) * 256])
                mv = small.tile([128, nc.vector.BN_AGGR_DIM], F32, tag="mv")
                nc.vector.bn_aggr(out=mv, in_=stats)
                rstd = small.tile([128, 1], F32, tag="rstd")
                nc.scalar.activation(out=rstd, in_=mv[:, 1:2], func=AF.Sqrt, bias=eps_t)
                nc.vector.reciprocal(out=rstd, in_=rstd)
                nc.vector.tensor_tensor(out=rstd, in0=rstd, in1=gate, op=ALU.mult)
                dst = scratch if accum else mix
                nc.vector.tensor_scalar(out=dst, in0=src_t[:, nt, :],
                                        scalar1=mv[:, 0:1], scalar2=rstd,
                                        op0=ALU.subtract, op1=ALU.mult)
                if accum:
                    nc.vector.tensor_tensor(out=mix, in0=mix, in1=scratch, op=ALU.add)
            for cc in range(CC):
                tp = pst("tp", [128, 128])
                nc.tensor.matmul(tp, mix[:, cc * 128:(cc + 1) * 128], ident_b,
                                 is_transpose=True, start=True, stop=True)
                nc.scalar.copy(out=mixT[:, nt, cc, :], in_=tp)
        prps = [pst("pr0", [128, 512]), pst("pr1", [128, 512])]
        out_sb = opool.tile([128, NT, C], F32, tag="outsb")
        for lo, hi in ((0, 512), (512, 768)):
            gw = hi - lo
            for cc in range(CC):
                w_t = wpool.tile([128, 512], BF16, tag="wpr")
                nc.sync.dma_start(
                    out=w_t[:, 0:gw],
                    in_=wprT.rearrange("(cc p) f -> p cc f", p=128)[:, cc, lo:hi])
                for nt in range(NT):
                    nc.tensor.matmul(prps[nt][:, 0:gw], mixT[:, nt, cc, :],
                                     w_t[:, 0:gw], start=(cc == 0), stop=False)
            for nt in range(NT):
                nc.tensor.matmul(prps[nt][:, 0:gw], ones_r_t,
                                 b_pr_t[0:1, lo:hi], start=False, stop=True)
                nc.scalar.copy(out=out_sb[:, nt, lo:hi], in_=prps[nt][:, 0:gw])
        for nt in range(NT):
            nc.sync.dma_start(out=out_nl[:, nt, :], in_=out_sb[:, nt, :])

    if repeat:
        with tc.For_i(0, repeat, 1):
            body()
    else:
        body()


def _build(repeat=0):
    nc = bacc.Bacc("TRN2", target_bir_lowering=False, debug=False, num_devices=8)

    def inp(name, shape, dtype=F32):
        return nc.dram_tensor(name, list(shape), dtype, kind="ExternalInput").ap()

    io = [
        inp("xT", (C, N), BF16),
        inp("wqkT", (C, 2 * C), BF16),
        inp("b_qk", (128, FT_QK)),
        inp("wvT", (C, C), BF16),
        inp("b_v", (C,), BF16),
        inp("wglT", (C, C), BF16),
        inp("b_gl", (128, CC)),
        inp("wloT", (C, C), BF16),
        inp("b_lo", (128, CC)),
        inp("wg2", (128, NPAIR * D), BF16),
        inp("wbgT", (128, D), BF16),
        inp("lam", (1, 1)),
        inp("wprT", (C, C), BF16),
        inp("b_pr", (C,), BF16),
        inp("ones_r", (1, 128), BF16),
        nc.dram_tensor("li_dram", [H, D * N], BF16).ap(),   # internal scratch
        nc.dram_tensor("out", [N, C], F32, kind="ExternalOutput").ap(),
    ]
    with tile.TileContext(nc) as tc:
        _core_kernel(tc, io, repeat=repeat)
    nc.compile()
    return nc


def kernel(**inputs):
    x = _f32(inputs["x"])
    w_qkv = _f32(inputs["w_qkv"]); b_qkv = _f32(inputs["b_qkv"])
    w_g = _f32(inputs["w_g"]); w_bg = _f32(inputs["w_bg"])
    w_local = _f32(inputs["w_local"]); b_local = _f32(inputs["b_local"])
    w_global = _f32(inputs["w_global"]); b_global = _f32(inputs["b_global"])
    lam = _f32(inputs["lam"])
    w_proj = _f32(inputs["w_proj"]); b_proj = _f32(inputs["b_proj"])

    wq = w_qkv[0:C] * SCALE
    wk = w_qkv[C:2 * C]
    wv = w_qkv[2 * C:3 * C]
    bq = b_qkv[0:C] * SCALE
    bk = b_qkv[C:2 * C]
    bv = b_qkv[2 * C:3 * C]
    wqkT = _bf16(np.concatenate([wq, wk], 0).T)
    b_qk = _f32(np.concatenate([bq, bk]).reshape(FT_QK, 128).T)
    wvT = _bf16(wv.T)
    wglT = _bf16(w_global.T)
    b_gl = _f32(b_global.reshape(CC, 128).T)
    wloT = _bf16(w_local.T)
    b_lo = _f32(b_local.reshape(CC, 128).T)
    wprT = _bf16(w_proj.T)
    wg3 = w_g.reshape(D, D, D)                # [d, e, c]
    wg2 = np.zeros((128, NPAIR * D), np.float32)
    for k in range(NPAIR):
        wg2[0:64, k * D:(k + 1) * D] = wg3[k].T
        wg2[64:128, k * D:(k + 1) * D] = wg3[k + NPAIR].T
    wg2 = _bf16(wg2)
    wbgT = _bf16(np.concatenate([w_bg.T, w_bg.T], 0))   # duplicated halves

    if "nc" not in _CACHED:
        _CACHED["nc"] = _build()
    nc = _CACHED["nc"]

    shared = dict(wqkT=wqkT, b_qk=b_qk, wvT=wvT, b_v=_bf16(bv), wglT=wglT, b_gl=b_gl,
                  wloT=wloT, b_lo=b_lo, wg2=wg2, wbgT=wbgT,
                  lam=lam.reshape(1, 1), wprT=wprT, b_pr=_bf16(b_proj),
                  ones_r=_bf16(np.ones((1, 128), np.float32)))
    in_maps = [dict(shared, xT=_bf16(x[b].T)) for b in range(B)]
    _CACHED["in_maps"] = in_maps
    res = run_bass_kernel_spmd(nc, in_maps, core_ids=list(range(B)))
    out = np.stack([res.results[b]["out"] for b in range(B)], 0)
    return out.astype(np.float32)


def _device_runner(nc, in_maps):
    """Single-bind sharded jitted fn with device-resident inputs."""
    import jax
    from jax.sharding import Mesh, PartitionSpec
    from jax.experimental.shard_map import shard_map
    import concourse.mybir as _mb
    from concourse import bass2jax as B2J

    B2J.install_neuronx_cc_hook()
    partition_name = nc.partition_id_tensor.name if nc.partition_id_tensor else None
    in_names, out_names, out_avals, zero_outs = [], [], [], []
    for alloc in nc.m.functions[0].allocations:
        if not isinstance(alloc, _mb.MemoryLocationSet):
            continue
        name = alloc.memorylocations[0].name
        if alloc.kind == "ExternalInput":
            if name != partition_name:
                in_names.append(name)
        elif alloc.kind == "ExternalOutput":
            shape = tuple(alloc.tensor_shape)
            dtype = _mb.dt.np(alloc.dtype)
            out_names.append(name)
            out_avals.append(jax.core.ShapedArray(shape, dtype))
            zero_outs.append(np.zeros(shape, dtype))
    n_params = len(in_names)
    all_in_names = list(in_names) + list(out_names)
    if partition_name is not None:
        all_in_names.append(partition_name)

    def _body(*args):
        operands = list(args)
        if partition_name is not None:
            operands.append(B2J.partition_id_tensor())
        return tuple(B2J._bass_exec_p.bind(
            *operands,
            out_avals=tuple(out_avals),
            in_names=tuple(all_in_names),
            out_names=tuple(out_names),
            lowering_input_output_aliases=(),
            sim_require_finite=True,
            sim_require_nnan=True,
            nc=nc,
        ))

    n_cores = len(in_maps)
    devices = jax.devices()[:n_cores]
    mesh = Mesh(np.asarray(devices), ("core",))
    n_outs = len(out_avals)
    sharded = jax.jit(shard_map(
        _body, mesh=mesh,
        in_specs=(PartitionSpec("core"),) * (n_params + n_outs),
        out_specs=(PartitionSpec("core"),) * n_outs, check_rep=False))
    per_core = [[np.asarray(m[nm]) for nm in in_names] for m in in_maps]
    concat_in = [np.concatenate([per_core[c][i] for c in range(n_cores)], 0)
                 for i in range(n_params)]
    concat_zeros = [np.zeros((n_cores * z.shape[0], *z.shape[1:]), z.dtype)
                    for z in zero_outs]
    dev_in = [jax.device_put(a) for a in concat_in]
    dev_zero = [jax.device_put(a) for a in concat_zeros]

    def run():
        return sharded(*dev_in, *dev_zero)

    return run


def run_timed(n_iters=1024, n_calls=12, **inputs):
    """Build repeat=1 and repeat=n_iters variants; time both with
    device-resident inputs; return estimated per-iteration ns
    (median-based; min-based printed for reference)."""
    import time
    import statistics
    import jax
    kernel(**inputs)
    in_maps = _CACHED["in_maps"]
    wmed, wmin = {}, {}
    for R in (1, n_iters):
        key = f"nc_rep{R}"
        if key not in _CACHED:
            _CACHED[key] = _build(repeat=R)
        run = _device_runner(_CACHED[key], in_maps)
        jax.block_until_ready(run())  # compile+warm
        ts = []
        for _ in range(n_calls):
            t0 = time.time()
            jax.block_until_ready(run())
            ts.append(time.time() - t0)
        wmed[R], wmin[R] = statistics.median(ts), min(ts)
        print(f"repeat={R}: med {wmed[R]*1e3:.2f} min {wmin[R]*1e3:.2f} ms "
              f"(all {[f'{t*1e3:.1f}' for t in ts]})")
    per_med = (wmed[n_iters] - wmed[1]) / (n_iters - 1) * 1e9
    per_min = (wmin[n_iters] - wmin[1]) / (n_iters - 1) * 1e9
    print(f"per-iter median-est {per_med:.0f} ns / min-est {per_min:.0f} ns")
    return per_med
